# revision 1
# baseline (speedup 1.0000x reference)
"""GCN encoder (3x gcn_conv) on 8 Trainium2 NeuronCores.

Graph-parallel by destination node:
- Nodes are sharded 6250/core; each core owns the edges whose destination
  (col) falls in its shard, grouped into destination blocks of 128 nodes.
- The layer-1 node table h1 = x @ W1 is built (replicated) on every core.
- Per 128-edge chunk: an indirect DMA gathers the 128 source rows h[row];
  the edge-attr linear runs on the tensor engine (K=8, bias folded in as a
  ones-row); relu on ACT; scatter-add is a one-hot matmul into a PSUM
  accumulator per destination block (S[e,dst] = (iota==colrel)*dinv_row,
  built on DVE), applying dinv[row]. dinv[col] and the self-loop term are
  applied per destination block.
- Layers mu/logstd share edges and gathers: their node tables are
  concatenated into one 128-wide table T2 = [h@Wmu | h@Wls], which is
  AllGathered across the cores between the two edge passes.
"""
import numpy as np

N_NODES = 50000
N_CORES = 8
SHARD = N_NODES // N_CORES          # 6250
P = 128
NBLK = (SHARD + P - 1) // P         # 49 destination blocks / core
IN_F = 128
HID = 128
OUT_F = 64
TBLK = (N_NODES + P - 1) // P       # 391 table-build chunks


def _host_prep(x, edge_index, edge_attr,
               W1, b1, We1, be1, root1,
               Wmu, bmu, Wemu, bemu, rootmu,
               Wls, bls, Wels, bels, rootls):
    x = np.asarray(x, np.float32)
    row = np.asarray(edge_index[0], np.int64)
    col = np.asarray(edge_index[1], np.int64)
    ea = np.asarray(edge_attr, np.float32)
    E = row.shape[0]

    deg = (np.bincount(row, minlength=N_NODES) + 1.0).astype(np.float32)
    dinv = deg ** -0.5
    rdeg = (1.0 / deg).astype(np.float32)

    core_of = col // SHARD
    blk_of = (col - core_of * SHARD) // P

    # uniform chunks-per-block across cores (SPMD: one program for all)
    counts = np.zeros((N_CORES, NBLK), np.int64)
    for c in range(N_CORES):
        m = core_of == c
        counts[c] = np.bincount(blk_of[m], minlength=NBLK)
    n_chunks = np.maximum(1, (counts.max(axis=0) + P - 1) // P).astype(int)
    NCH = int(n_chunks.sum())

    offs = np.zeros((N_CORES, P, NCH), np.int32)
    colrel = np.full((N_CORES, P, NCH), -1.0, np.float32)
    srw = np.zeros((N_CORES, P, NCH), np.float32)
    at = np.zeros((N_CORES, 8, NCH * P), np.float32)
    chunk_base = np.concatenate([[0], np.cumsum(n_chunks)])[:-1]

    order = np.lexsort((blk_of, core_of))
    row_s, col_s = row[order], col[order]
    core_s, blk_s = core_of[order], blk_of[order]
    ea_s = ea[order]
    seg_cnt = np.zeros(N_CORES * NBLK + 1, np.int64)
    np.add.at(seg_cnt, core_s * NBLK + blk_s + 1, 1)
    seg_start = np.cumsum(seg_cnt)
    pos_in_seg = np.arange(E) - seg_start[core_s * NBLK + blk_s]

    chunk_idx = chunk_base[blk_s] + pos_in_seg // P
    part_idx = pos_in_seg % P

    offs[core_s, part_idx, chunk_idx] = row_s.astype(np.int32)
    colrel[core_s, part_idx, chunk_idx] = (col_s - core_s * SHARD - blk_s * P).astype(np.float32)
    srw[core_s, part_idx, chunk_idx] = dinv[row_s]
    flat = chunk_idx * P + part_idx
    for j in range(7):
        at[core_s, j, flat] = ea_s[:, j]
    at[core_s, 7, flat] = 1.0

    dinvcol = np.zeros((N_CORES, P, NBLK), np.float32)
    rdegc = np.zeros((N_CORES, P, NBLK), np.float32)
    selfoff = np.zeros((N_CORES, P, NBLK), np.int32)
    for c in range(N_CORES):
        ids = c * SHARD + np.arange(SHARD)
        b = np.arange(SHARD) // P
        p = np.arange(SHARD) % P
        dinvcol[c, p, b] = dinv[ids]
        rdegc[c, p, b] = rdeg[ids]
        selfoff[c, p, b] = ids

    W1 = np.asarray(W1, np.float32)
    we1 = np.concatenate([np.asarray(We1, np.float32),
                          (np.asarray(be1) + np.asarray(b1))[None, :]], 0).astype(np.float32)
    bias1 = np.tile((np.asarray(b1) + np.asarray(root1))[None, :], (P, 1)).astype(np.float32)
    wcat = np.concatenate([np.asarray(Wmu), np.asarray(Wls)], 1).astype(np.float32)
    we2 = np.concatenate([
        np.concatenate([np.asarray(Wemu), np.asarray(Wels)], 1),
        np.concatenate([np.asarray(bemu) + np.asarray(bmu),
                        np.asarray(bels) + np.asarray(bls)])[None, :]], 0).astype(np.float32)
    bias2 = np.tile(np.concatenate([np.asarray(bmu) + np.asarray(rootmu),
                                    np.asarray(bls) + np.asarray(rootls)])[None, :],
                    (P, 1)).astype(np.float32)
    iota = np.tile(np.arange(P, dtype=np.float32)[None, :], (P, 1))
    ident = np.eye(P, dtype=np.float32)
    xrows = np.ascontiguousarray(x)

    shared = dict(xrows=xrows, W1=W1, we1=we1, bias1=bias1, wcat=wcat, we2=we2,
                  bias2=bias2, iota=iota, ident=ident)
    # layer-2 gather offsets against the split hfull layout:
    # rows [0, 8*HALF) = ranks' first halves; rows [8*HALF, N) = second halves
    HALF = SHARD // 2
    oc = offs // SHARD          # owning core of each source row
    orr = offs - oc * SHARD     # local row within that core's shard
    offs2 = np.where(orr < HALF, oc * HALF + orr,
                     N_CORES * HALF + oc * (SHARD - HALF) + (orr - HALF)).astype(np.int32)
    per_core = []
    for c in range(N_CORES):
        d = dict(offs=offs[c], offs2=offs2[c], colrel=colrel[c], srw=srw[c],
                 at=at[c], dinvcol=dinvcol[c], rdegc=rdegc[c],
                 xself=np.ascontiguousarray(x[c * SHARD:(c + 1) * SHARD]))
        d.update(shared)
        per_core.append(d)
    return per_core, n_chunks, NCH


HALF = SHARD // 2


def _build_nc(n_chunks, NCH, phases=("tab","self","l1","ag","l2"), max_sched=None, ablate=()):
    from concourse import bass, bacc, mybir
    import concourse.tile as tile

    f32 = mybir.dt.float32
    i32 = mybir.dt.int32
    Relu = mybir.ActivationFunctionType.Relu
    Alu = mybir.AluOpType
    nc = bacc.Bacc(None, num_devices=N_CORES)

    xrows_d = nc.declare_dram_parameter("xrows", [N_NODES, IN_F], f32, isOutput=False)
    W1_d = nc.declare_dram_parameter("W1", [IN_F, HID], f32, isOutput=False)
    we1_d = nc.declare_dram_parameter("we1", [8, HID], f32, isOutput=False)
    bias1_d = nc.declare_dram_parameter("bias1", [P, HID], f32, isOutput=False)
    wcat_d = nc.declare_dram_parameter("wcat", [HID, P], f32, isOutput=False)
    we2_d = nc.declare_dram_parameter("we2", [8, P], f32, isOutput=False)
    bias2_d = nc.declare_dram_parameter("bias2", [P, P], f32, isOutput=False)
    iota_d = nc.declare_dram_parameter("iota", [P, P], f32, isOutput=False)
    ident_d = nc.declare_dram_parameter("ident", [P, P], f32, isOutput=False)
    offs_d = nc.declare_dram_parameter("offs", [P, NCH], i32, isOutput=False)
    offs2_d = nc.declare_dram_parameter("offs2", [P, NCH], i32, isOutput=False)
    colrel_d = nc.declare_dram_parameter("colrel", [P, NCH], f32, isOutput=False)
    srw_d = nc.declare_dram_parameter("srw", [P, NCH], f32, isOutput=False)
    at_d = nc.declare_dram_parameter("at", [8, NCH * P], f32, isOutput=False)
    dinvcol_d = nc.declare_dram_parameter("dinvcol", [P, NBLK], f32, isOutput=False)
    rdegc_d = nc.declare_dram_parameter("rdegc", [P, NBLK], f32, isOutput=False)
    xself_d = nc.declare_dram_parameter("xself", [SHARD, IN_F], f32, isOutput=False)
    out_d = nc.declare_dram_parameter("out", [SHARD, P], f32, isOutput=True)

    hshardA = nc.dram_tensor("hshardA", [HALF, HID], f32)
    hshardB = nc.dram_tensor("hshardB", [SHARD - HALF, HID], f32)
    hfull = nc.dram_tensor("hfull", [N_NODES, HID], f32, addr_space="Shared")

    SUP = 4      # chunks per elementwise batch
    ATSUP = 64   # chunks per edge-attr stream tile

    sched = []
    for b, nk in enumerate(n_chunks):
        for k in range(nk):
            sched.append((b, k, int(nk)))
    if max_sched is not None:
        # truncate to whole blocks
        sched = [t for t in sched if t[0] < max_sched]
        phases = tuple(phases)

    with tile.TileContext(nc) as tc:
        with (
            tc.tile_pool(name="const", bufs=1) as cpool,
            tc.tile_pool(name="selfb", bufs=1) as spool,
            tc.tile_pool(name="stream", bufs=2) as stpool,
            tc.tile_pool(name="work", bufs=3) as wpool,
            tc.tile_pool(name="node", bufs=3) as npool,
            tc.tile_pool(name="pse", bufs=2, space="PSUM") as pse,
            tc.tile_pool(name="psagg", bufs=2, space="PSUM") as psagg,
            tc.tile_pool(name="psnode", bufs=2, space="PSUM") as psnode,
        ):
            W1_t = cpool.tile([IN_F, HID], f32)
            we1_t = cpool.tile([8, HID], f32)
            bias1_t = cpool.tile([P, HID], f32)
            wcat_t = cpool.tile([HID, P], f32)
            we2_t = cpool.tile([8, P], f32)
            bias2_t = cpool.tile([P, P], f32)
            iota_t = cpool.tile([P, P], f32)
            ident_t = cpool.tile([P, P], f32)
            offs_t = cpool.tile([P, NCH], i32)
            offs2_t = cpool.tile([P, NCH], i32)
            colrel_t = cpool.tile([P, NCH], f32)
            srw_t = cpool.tile([P, NCH], f32)
            dinvcol_t = cpool.tile([P, NBLK], f32)
            rdegc_t = cpool.tile([P, NBLK], f32)
            for t, d in ((W1_t, W1_d), (we1_t, we1_d), (bias1_t, bias1_d),
                         (wcat_t, wcat_d), (we2_t, we2_d), (bias2_t, bias2_d),
                         (iota_t, iota_d), (ident_t, ident_d), (offs_t, offs_d),
                         (offs2_t, offs2_d),
                         (colrel_t, colrel_d), (srw_t, srw_d),
                         (dinvcol_t, dinvcol_d), (rdegc_t, rdegc_d)):
                nc.sync.dma_start(out=t[:], in_=d[:])

            selfbuf1 = [spool.tile([P, HID], f32, name=f"s1_{b}", tag=f"s1_{b}") for b in range(NBLK)]
            selfbuf2 = [spool.tile([P, P], f32, name=f"s2_{b}", tag=f"s2_{b}") for b in range(NBLK)]

            # ---- phase 1b: self term, own shard: relu(x@W1 + b1 + root1) ----
            for b in range(NBLK if "self" in phases else 0):
                xrows = npool.tile([P, HID], f32, tag="xrows")
                lo_s = b * P
                n_s = min(P, SHARD - lo_s)
                nc.sync.dma_start(out=xrows[:n_s, :], in_=xself_d[lo_s:lo_s + n_s, :])
                psx = psnode.tile([P, P], f32, tag="pn")
                nc.tensor.transpose(out=psx[:], in_=xrows[:], identity=ident_t[:])
                xgT = npool.tile([P, P], f32, tag="xgT")
                nc.vector.tensor_copy(out=xgT[:], in_=psx[:])
                psh = psnode.tile([P, HID], f32, tag="pn")
                nc.tensor.matmul(out=psh[:], lhsT=xgT[:], rhs=W1_t[:], start=True, stop=True)
                pre = npool.tile([P, HID], f32, tag="pre1")
                nc.vector.tensor_tensor(out=pre[:], in0=psh[:], in1=bias1_t[:], op=Alu.add)
                nc.scalar.activation(selfbuf1[b][:], pre[:], Relu)

            # ---- edge pass helper ----
            def edge_pass(layer, table, weaug_t, wtab_t, block_done, eoffs_t):
                at_tile = [None]
                cur_at = [-1]
                sup = {}
                pend = []

                def flush(nq):
                    nc.scalar.activation(sup["msg"][:, :nq, :], sup["eps"][:, :nq, :], Relu)
                    for (qq, bb, kk, nkk, agg) in pend:
                        nc.tensor.matmul(
                            out=agg[:], lhsT=sup["S"][:, qq, :],
                            rhs=sup["msg"][:, qq, :],
                            start=(kk == 0), stop=(kk == nkk - 1))
                        if kk == nkk - 1:
                            block_done(bb, agg)
                    pend.clear()

                agg = None
                for cidx, (b, k, nk) in enumerate(sched):
                    q = cidx % SUP
                    if q == 0:
                        sup["g"] = wpool.tile([P, SUP, P], f32, name=f"g{layer}", tag=f"g{layer}")
                        sup["eps"] = pse.tile([P, SUP, P], f32, name="eps", tag="eps")
                        sup["S"] = wpool.tile([P, SUP, P], f32, name=f"S{layer}", tag=f"S{layer}")
                        sup["msg"] = wpool.tile([P, SUP, P], f32, name=f"msg{layer}", tag=f"msg{layer}")
                    if cidx // ATSUP != cur_at[0]:
                        cur_at[0] = cidx // ATSUP
                        lo = cur_at[0] * ATSUP * P
                        n = min(ATSUP * P, NCH * P - lo)
                        at_tile[0] = stpool.tile([8, ATSUP * P], f32, name="at", tag="at")
                        nc.sync.dma_start(out=at_tile[0][:, :n], in_=at_d[:, lo:lo + n])
                    if k == 0:
                        agg = psagg.tile([P, P], f32, tag="agg")
                    if "gather" not in ablate:
                        nc.gpsimd.indirect_dma_start(
                            out=sup["g"][:, q, :], out_offset=None, in_=table[:],
                            in_offset=bass.IndirectOffsetOnAxis(
                                ap=eoffs_t[:, cidx:cidx + 1], axis=0))
                    else:
                        nc.sync.dma_start(out=sup["g"][:, q, :], in_=table[0:P, :])
                    a0 = (cidx - cur_at[0] * ATSUP) * P
                    pst = psnode.tile([P, P], f32, tag="pn", name="pst")
                    nc.tensor.transpose(out=pst[:], in_=sup["g"][:, q, :],
                                        identity=ident_t[:])
                    gT = wpool.tile([P, P], f32, tag=f"gT{layer}", name="gT")
                    nc.vector.tensor_copy(out=gT[:], in_=pst[:])
                    nc.tensor.matmul(out=sup["eps"][:, q, :],
                                     lhsT=at_tile[0][:, a0:a0 + P],
                                     rhs=weaug_t[:], start=True, stop=False)
                    nc.tensor.matmul(out=sup["eps"][:, q, :],
                                     lhsT=gT[:], rhs=wtab_t[:],
                                     start=False, stop=True)
                    if "sgen" not in ablate:
                        nc.vector.tensor_scalar(
                            out=sup["S"][:, q, :], in0=iota_t[:],
                            scalar1=colrel_t[:, cidx:cidx + 1],
                            scalar2=srw_t[:, cidx:cidx + 1],
                            op0=Alu.is_equal, op1=Alu.mult)
                    else:
                        nc.gpsimd.memset(sup["S"][:, q, :], 0.0)
                    pend.append((q, b, k, nk, agg))
                    if q == SUP - 1 or cidx == len(sched) - 1:
                        flush(q + 1)

            # ---- phase 2: layer-1 edge pass; block finals build T2 ----
            def l1_block_done(b, agg):
                u = npool.tile([P, HID], f32, tag="u")
                nc.vector.tensor_scalar(out=u[:], in0=agg[:],
                                        scalar1=dinvcol_t[:, b:b + 1], scalar2=None,
                                        op0=Alu.mult)
                v = npool.tile([P, HID], f32, tag="v")
                nc.vector.tensor_scalar(out=v[:], in0=selfbuf1[b][:],
                                        scalar1=rdegc_t[:, b:b + 1], scalar2=None,
                                        op0=Alu.mult)
                w = npool.tile([P, HID], f32, tag="w")
                nc.vector.tensor_tensor(out=w[:], in0=u[:], in1=v[:], op=Alu.add)
                hb = npool.tile([P, HID], f32, tag="hb")
                nc.scalar.activation(hb[:], w[:], Relu)
                lo = b * P
                n = min(P, SHARD - lo)
                if lo + n <= HALF:
                    nc.sync.dma_start(out=hshardA[lo:lo + n, :], in_=hb[:n, :])
                elif lo >= HALF:
                    nc.sync.dma_start(out=hshardB[lo - HALF:lo - HALF + n, :], in_=hb[:n, :])
                else:
                    nA = HALF - lo
                    nc.sync.dma_start(out=hshardA[lo:HALF, :], in_=hb[:nA, :])
                    nc.sync.dma_start(out=hshardB[0:lo + n - HALF, :], in_=hb[nA:n, :])
                pst = psnode.tile([P, P], f32, tag="pn")
                nc.tensor.transpose(out=pst[:], in_=hb[:], identity=ident_t[:])
                hT = npool.tile([P, P], f32, tag="hT")
                nc.vector.tensor_copy(out=hT[:], in_=pst[:])
                ps2 = psnode.tile([P, P], f32, tag="pn")
                nc.tensor.matmul(out=ps2[:], lhsT=hT[:], rhs=wcat_t[:], start=True, stop=True)
                pre2 = npool.tile([P, P], f32, tag="pre2")
                nc.vector.tensor_tensor(out=pre2[:], in0=ps2[:], in1=bias2_t[:], op=Alu.add)
                nc.scalar.activation(selfbuf2[b][:], pre2[:], Relu)

            if "l1" in phases:
                edge_pass(1, xrows_d, we1_t, W1_t, l1_block_done, offs_t)

            # ---- phase 3: all-gather T2 ----
            if "ag" in phases:
                nc.gpsimd.collective_compute(
                    "AllGather", mybir.AluOpType.bypass,
                    replica_groups=[list(range(N_CORES))],
                    ins=[hshardA[:]], outs=[hfull[0:N_CORES * HALF, :]])
                nc.gpsimd.collective_compute(
                    "AllGather", mybir.AluOpType.bypass,
                    replica_groups=[list(range(N_CORES))],
                    ins=[hshardB[:]], outs=[hfull[N_CORES * HALF:, :]])

            # ---- phase 4: layer-2/3 edge pass ----
            def l2_block_done(b, agg):
                u = npool.tile([P, P], f32, tag="u2")
                nc.vector.tensor_scalar(out=u[:], in0=agg[:],
                                        scalar1=dinvcol_t[:, b:b + 1], scalar2=None,
                                        op0=Alu.mult)
                v = npool.tile([P, P], f32, tag="v2")
                nc.vector.tensor_scalar(out=v[:], in0=selfbuf2[b][:],
                                        scalar1=rdegc_t[:, b:b + 1], scalar2=None,
                                        op0=Alu.mult)
                w = npool.tile([P, P], f32, tag="w2")
                nc.vector.tensor_tensor(out=w[:], in0=u[:], in1=v[:], op=Alu.add)
                lo = b * P
                n = min(P, SHARD - lo)
                nc.sync.dma_start(out=out_d[lo:lo + n, :], in_=w[:n, :])

            if "l2" in phases:
                edge_pass(2, hfull, we2_t, wcat_t, l2_block_done, offs2_t)
            else:
                ztmp = npool.tile([P, P], f32, tag="ztmp")
                nc.vector.memset(ztmp[:], 0.0)
                nc.sync.dma_start(out=out_d[0:P, :], in_=ztmp[:])

    nc.finalize()
    return nc


_CACHE = {}


def kernel(**inputs):
    from concourse.bass_utils import run_bass_kernel_spmd

    per_core, n_chunks, NCH = _host_prep(**inputs)
    key = (tuple(n_chunks), NCH)
    if key not in _CACHE:
        _CACHE[key] = _build_nc(n_chunks, NCH)
    nc = _CACHE[key]
    r = None
    for attempt in range(3):
        try:
            r = run_bass_kernel_spmd(nc, per_core, list(range(N_CORES)))
            break
        except Exception:
            if attempt == 2:
                raise
            import time as _time
            _time.sleep(5.0)
    outs = [r.results[c]["out"] for c in range(N_CORES)]
    full = np.concatenate(outs, axis=0)
    mu = np.ascontiguousarray(full[:, :OUT_F])
    logstd = np.ascontiguousarray(full[:, OUT_F:])
    return (mu, logstd)



# revision 7
# speedup vs baseline: 1.0009x; 1.0009x over previous
"""GCN encoder (3x gcn_conv) on 8 Trainium2 NeuronCores.

Graph-parallel by destination node:
- Nodes are sharded 6250/core; each core owns the edges whose destination
  (col) falls in its shard, grouped into destination blocks of 128 nodes.
- The layer-1 node table h1 = x @ W1 is built (replicated) on every core.
- Per 128-edge chunk: an indirect DMA gathers the 128 source rows h[row];
  the edge-attr linear runs on the tensor engine (K=8, bias folded in as a
  ones-row); relu on ACT; scatter-add is a one-hot matmul into a PSUM
  accumulator per destination block (S[e,dst] = (iota==colrel)*dinv_row,
  built on DVE), applying dinv[row]. dinv[col] and the self-loop term are
  applied per destination block.
- Layers mu/logstd share edges and gathers: their node tables are
  concatenated into one 128-wide table T2 = [h@Wmu | h@Wls], which is
  AllGathered across the cores between the two edge passes.
"""
import numpy as np

N_NODES = 50000
N_CORES = 8
SHARD = N_NODES // N_CORES          # 6250
P = 128
NBLK = (SHARD + P - 1) // P         # 49 destination blocks / core
IN_F = 128
HID = 128
OUT_F = 64
TBLK = (N_NODES + P - 1) // P       # 391 table-build chunks


def _host_prep(x, edge_index, edge_attr,
               W1, b1, We1, be1, root1,
               Wmu, bmu, Wemu, bemu, rootmu,
               Wls, bls, Wels, bels, rootls):
    x = np.asarray(x, np.float32)
    row = np.asarray(edge_index[0], np.int64)
    col = np.asarray(edge_index[1], np.int64)
    ea = np.asarray(edge_attr, np.float32)
    E = row.shape[0]

    deg = (np.bincount(row, minlength=N_NODES) + 1.0).astype(np.float32)
    dinv = deg ** -0.5
    rdeg = (1.0 / deg).astype(np.float32)

    core_of = col // SHARD
    blk_of = (col - core_of * SHARD) // P

    # uniform chunks-per-block across cores (SPMD: one program for all)
    counts = np.zeros((N_CORES, NBLK), np.int64)
    for c in range(N_CORES):
        m = core_of == c
        counts[c] = np.bincount(blk_of[m], minlength=NBLK)
    n_chunks = np.maximum(1, (counts.max(axis=0) + P - 1) // P).astype(int)
    NCH = int(n_chunks.sum())

    offs = np.zeros((N_CORES, P, NCH), np.int32)
    colrel = np.full((N_CORES, P, NCH), -1.0, np.float32)
    srw = np.zeros((N_CORES, P, NCH), np.float32)
    at = np.zeros((N_CORES, 8, NCH * P), np.float32)
    chunk_base = np.concatenate([[0], np.cumsum(n_chunks)])[:-1]

    order = np.lexsort((blk_of, core_of))
    row_s, col_s = row[order], col[order]
    core_s, blk_s = core_of[order], blk_of[order]
    ea_s = ea[order]
    seg_cnt = np.zeros(N_CORES * NBLK + 1, np.int64)
    np.add.at(seg_cnt, core_s * NBLK + blk_s + 1, 1)
    seg_start = np.cumsum(seg_cnt)
    pos_in_seg = np.arange(E) - seg_start[core_s * NBLK + blk_s]

    chunk_idx = chunk_base[blk_s] + pos_in_seg // P
    part_idx = pos_in_seg % P

    offs[core_s, part_idx, chunk_idx] = row_s.astype(np.int32)
    colrel[core_s, part_idx, chunk_idx] = (col_s - core_s * SHARD - blk_s * P).astype(np.float32)
    srw[core_s, part_idx, chunk_idx] = dinv[row_s]
    flat = chunk_idx * P + part_idx
    for j in range(7):
        at[core_s, j, flat] = ea_s[:, j]
    at[core_s, 7, flat] = 1.0

    dinvcol = np.zeros((N_CORES, P, NBLK), np.float32)
    rdegc = np.zeros((N_CORES, P, NBLK), np.float32)
    selfoff = np.zeros((N_CORES, P, NBLK), np.int32)
    for c in range(N_CORES):
        ids = c * SHARD + np.arange(SHARD)
        b = np.arange(SHARD) // P
        p = np.arange(SHARD) % P
        dinvcol[c, p, b] = dinv[ids]
        rdegc[c, p, b] = rdeg[ids]
        selfoff[c, p, b] = ids

    W1 = np.asarray(W1, np.float32)
    we1 = np.concatenate([np.asarray(We1, np.float32),
                          (np.asarray(be1) + np.asarray(b1))[None, :]], 0).astype(np.float32)
    bias1 = np.tile((np.asarray(b1) + np.asarray(root1))[None, :], (P, 1)).astype(np.float32)
    wcat = np.concatenate([np.asarray(Wmu), np.asarray(Wls)], 1).astype(np.float32)
    we2 = np.concatenate([
        np.concatenate([np.asarray(Wemu), np.asarray(Wels)], 1),
        np.concatenate([np.asarray(bemu) + np.asarray(bmu),
                        np.asarray(bels) + np.asarray(bls)])[None, :]], 0).astype(np.float32)
    bias2 = np.tile(np.concatenate([np.asarray(bmu) + np.asarray(rootmu),
                                    np.asarray(bls) + np.asarray(rootls)])[None, :],
                    (P, 1)).astype(np.float32)
    iota = np.tile(np.arange(P, dtype=np.float32)[None, :], (P, 1))
    ident = np.eye(P, dtype=np.float32)
    xrows = np.ascontiguousarray(x)

    shared = dict(xrows=xrows, W1=W1, we1=we1, bias1=bias1, wcat=wcat, we2=we2,
                  bias2=bias2, iota=iota, ident=ident)
    # layer-2 gather offsets against the split hfull layout:
    # rows [0, 8*HALF) = ranks' first halves; rows [8*HALF, N) = second halves
    HALF = SHARD // 2
    oc = offs // SHARD          # owning core of each source row
    orr = offs - oc * SHARD     # local row within that core's shard
    offs2 = np.where(orr < HALF, oc * HALF + orr,
                     N_CORES * HALF + oc * (SHARD - HALF) + (orr - HALF)).astype(np.int32)
    per_core = []
    for c in range(N_CORES):
        d = dict(offs=offs[c], offs2=offs2[c], colrel=colrel[c], srw=srw[c],
                 at=at[c], dinvcol=dinvcol[c], rdegc=rdegc[c],
                 xself=np.ascontiguousarray(x[c * SHARD:(c + 1) * SHARD]))
        d.update(shared)
        per_core.append(d)
    return per_core, n_chunks, NCH


HALF = SHARD // 2


def _build_nc(n_chunks, NCH, phases=("tab","self","l1","ag","l2"), max_sched=None, ablate=()):
    from concourse import bass, bacc, mybir
    import concourse.tile as tile

    f32 = mybir.dt.float32
    i32 = mybir.dt.int32
    Relu = mybir.ActivationFunctionType.Relu
    Alu = mybir.AluOpType
    nc = bacc.Bacc(None, num_devices=N_CORES)

    xrows_d = nc.declare_dram_parameter("xrows", [N_NODES, IN_F], f32, isOutput=False)
    W1_d = nc.declare_dram_parameter("W1", [IN_F, HID], f32, isOutput=False)
    we1_d = nc.declare_dram_parameter("we1", [8, HID], f32, isOutput=False)
    bias1_d = nc.declare_dram_parameter("bias1", [P, HID], f32, isOutput=False)
    wcat_d = nc.declare_dram_parameter("wcat", [HID, P], f32, isOutput=False)
    we2_d = nc.declare_dram_parameter("we2", [8, P], f32, isOutput=False)
    bias2_d = nc.declare_dram_parameter("bias2", [P, P], f32, isOutput=False)
    iota_d = nc.declare_dram_parameter("iota", [P, P], f32, isOutput=False)
    ident_d = nc.declare_dram_parameter("ident", [P, P], f32, isOutput=False)
    offs_d = nc.declare_dram_parameter("offs", [P, NCH], i32, isOutput=False)
    offs2_d = nc.declare_dram_parameter("offs2", [P, NCH], i32, isOutput=False)
    colrel_d = nc.declare_dram_parameter("colrel", [P, NCH], f32, isOutput=False)
    srw_d = nc.declare_dram_parameter("srw", [P, NCH], f32, isOutput=False)
    at_d = nc.declare_dram_parameter("at", [8, NCH * P], f32, isOutput=False)
    dinvcol_d = nc.declare_dram_parameter("dinvcol", [P, NBLK], f32, isOutput=False)
    rdegc_d = nc.declare_dram_parameter("rdegc", [P, NBLK], f32, isOutput=False)
    xself_d = nc.declare_dram_parameter("xself", [SHARD, IN_F], f32, isOutput=False)
    out_d = nc.declare_dram_parameter("out", [SHARD, P], f32, isOutput=True)

    hshardA = nc.dram_tensor("hshardA", [HALF, HID], f32)
    hshardB = nc.dram_tensor("hshardB", [SHARD - HALF, HID], f32)
    hfull = nc.dram_tensor("hfull", [N_NODES, HID], f32, addr_space="Shared")

    SUP = 4      # chunks per elementwise batch
    ATSUP = 64   # chunks per edge-attr stream tile

    sched = []
    for b, nk in enumerate(n_chunks):
        for k in range(nk):
            sched.append((b, k, int(nk)))
    if max_sched is not None:
        # truncate to whole blocks
        sched = [t for t in sched if t[0] < max_sched]
        phases = tuple(phases)

    with tile.TileContext(nc) as tc:
        with (
            tc.tile_pool(name="const", bufs=1) as cpool,
            tc.tile_pool(name="selfb", bufs=1) as spool,
            tc.tile_pool(name="stream", bufs=2) as stpool,
            tc.tile_pool(name="work", bufs=3) as wpool,
            tc.tile_pool(name="node", bufs=3) as npool,
            tc.tile_pool(name="pse", bufs=2, space="PSUM") as pse,
            tc.tile_pool(name="psagg", bufs=2, space="PSUM") as psagg,
            tc.tile_pool(name="psnode", bufs=2, space="PSUM") as psnode,
        ):
            W1_t = cpool.tile([IN_F, HID], f32)
            we1_t = cpool.tile([8, HID], f32)
            bias1_t = cpool.tile([P, HID], f32)
            wcat_t = cpool.tile([HID, P], f32)
            we2_t = cpool.tile([8, P], f32)
            bias2_t = cpool.tile([P, P], f32)
            iota_t = cpool.tile([P, P], f32)
            ident_t = cpool.tile([P, P], f32)
            offs_t = cpool.tile([P, NCH], i32)
            offs2_t = cpool.tile([P, NCH], i32)
            colrel_t = cpool.tile([P, NCH], f32)
            srw_t = cpool.tile([P, NCH], f32)
            dinvcol_t = cpool.tile([P, NBLK], f32)
            rdegc_t = cpool.tile([P, NBLK], f32)
            for t, d in ((W1_t, W1_d), (we1_t, we1_d), (bias1_t, bias1_d),
                         (wcat_t, wcat_d), (we2_t, we2_d), (bias2_t, bias2_d),
                         (iota_t, iota_d), (ident_t, ident_d), (offs_t, offs_d),
                         (offs2_t, offs2_d),
                         (colrel_t, colrel_d), (srw_t, srw_d),
                         (dinvcol_t, dinvcol_d), (rdegc_t, rdegc_d)):
                nc.sync.dma_start(out=t[:], in_=d[:])

            selfbuf1 = [spool.tile([P, HID], f32, name=f"s1_{b}", tag=f"s1_{b}") for b in range(NBLK)]
            selfbuf2 = [spool.tile([P, P], f32, name=f"s2_{b}", tag=f"s2_{b}") for b in range(NBLK)]

            # ---- phase 1b: self term, own shard: relu(x@W1 + b1 + root1) ----
            for b in range(NBLK if "self" in phases else 0):
                xrows = npool.tile([P, HID], f32, tag="xrows")
                lo_s = b * P
                n_s = min(P, SHARD - lo_s)
                nc.sync.dma_start(out=xrows[:n_s, :], in_=xself_d[lo_s:lo_s + n_s, :])
                psx = psnode.tile([P, P], f32, tag="pn")
                nc.tensor.transpose(out=psx[:], in_=xrows[:], identity=ident_t[:])
                xgT = npool.tile([P, P], f32, tag="xgT")
                nc.vector.tensor_copy(out=xgT[:], in_=psx[:])
                psh = psnode.tile([P, HID], f32, tag="pn")
                nc.tensor.matmul(out=psh[:], lhsT=xgT[:], rhs=W1_t[:], start=True, stop=True)
                pre = npool.tile([P, HID], f32, tag="pre1")
                nc.vector.tensor_tensor(out=pre[:], in0=psh[:], in1=bias1_t[:], op=Alu.add)
                nc.scalar.activation(selfbuf1[b][:], pre[:], Relu)

            # ---- edge pass helper ----
            def edge_pass(layer, table, weaug_t, wtab_t, block_done, eoffs_t):
                at_tile = [None]
                cur_at = [-1]
                sup = {}
                pend = []

                def flush(nq):
                    nc.scalar.activation(sup["msg"][:, :nq, :], sup["eps"][:, :nq, :], Relu)
                    for (qq, bb, kk, nkk, agg) in pend:
                        nc.tensor.matmul(
                            out=agg[:], lhsT=sup["S"][:, qq, :],
                            rhs=sup["msg"][:, qq, :],
                            start=(kk == 0), stop=(kk == nkk - 1))
                        if kk == nkk - 1:
                            block_done(bb, agg)
                    pend.clear()

                agg = None
                for cidx, (b, k, nk) in enumerate(sched):
                    q = cidx % SUP
                    if q == 0:
                        sup["g"] = wpool.tile([P, SUP, P], f32, name=f"g{layer}", tag=f"g{layer}")
                        sup["eps"] = pse.tile([P, SUP, P], f32, name="eps", tag="eps")
                        sup["S"] = wpool.tile([P, SUP, P], f32, name=f"S{layer}", tag=f"S{layer}")
                        sup["msg"] = wpool.tile([P, SUP, P], f32, name=f"msg{layer}", tag=f"msg{layer}")
                    if cidx // ATSUP != cur_at[0]:
                        cur_at[0] = cidx // ATSUP
                        lo = cur_at[0] * ATSUP * P
                        n = min(ATSUP * P, NCH * P - lo)
                        at_tile[0] = stpool.tile([8, ATSUP * P], f32, name="at", tag="at")
                        nc.sync.dma_start(out=at_tile[0][:, :n], in_=at_d[:, lo:lo + n])
                    if k == 0:
                        agg = psagg.tile([P, P], f32, tag="agg")
                    if "gather" not in ablate:
                        nc.gpsimd.indirect_dma_start(
                            out=sup["g"][:, q, :], out_offset=None, in_=table[:],
                            in_offset=bass.IndirectOffsetOnAxis(
                                ap=eoffs_t[:, cidx:cidx + 1], axis=0))
                    else:
                        nc.sync.dma_start(out=sup["g"][:, q, :], in_=table[0:P, :])
                    a0 = (cidx - cur_at[0] * ATSUP) * P
                    pst = psnode.tile([P, P], f32, tag="pn", name="pst")
                    nc.tensor.transpose(out=pst[:], in_=sup["g"][:, q, :],
                                        identity=ident_t[:])
                    gT = wpool.tile([P, P], f32, tag=f"gT{layer}", name="gT")
                    nc.vector.tensor_copy(out=gT[:], in_=pst[:])
                    nc.tensor.matmul(out=sup["eps"][:, q, :],
                                     lhsT=at_tile[0][:, a0:a0 + P],
                                     rhs=weaug_t[:], start=True, stop=False)
                    nc.tensor.matmul(out=sup["eps"][:, q, :],
                                     lhsT=gT[:], rhs=wtab_t[:],
                                     start=False, stop=True)
                    if "sgen" not in ablate:
                        nc.vector.tensor_scalar(
                            out=sup["S"][:, q, :], in0=iota_t[:],
                            scalar1=colrel_t[:, cidx:cidx + 1],
                            scalar2=srw_t[:, cidx:cidx + 1],
                            op0=Alu.is_equal, op1=Alu.mult)
                    else:
                        nc.gpsimd.memset(sup["S"][:, q, :], 0.0)
                    pend.append((q, b, k, nk, agg))
                    if q == SUP - 1 or cidx == len(sched) - 1:
                        flush(q + 1)

            # ---- phase 2: layer-1 edge pass; block finals build T2 ----
            def l1_block_done(b, agg):
                u = npool.tile([P, HID], f32, tag="u")
                nc.vector.tensor_scalar(out=u[:], in0=agg[:],
                                        scalar1=dinvcol_t[:, b:b + 1], scalar2=None,
                                        op0=Alu.mult)
                v = npool.tile([P, HID], f32, tag="v")
                nc.vector.tensor_scalar(out=v[:], in0=selfbuf1[b][:],
                                        scalar1=rdegc_t[:, b:b + 1], scalar2=None,
                                        op0=Alu.mult)
                w = npool.tile([P, HID], f32, tag="w")
                nc.vector.tensor_tensor(out=w[:], in0=u[:], in1=v[:], op=Alu.add)
                hb = npool.tile([P, HID], f32, tag="hb")
                nc.scalar.activation(hb[:], w[:], Relu)
                lo = b * P
                n = min(P, SHARD - lo)
                if lo + n <= HALF:
                    nc.sync.dma_start(out=hshardA[lo:lo + n, :], in_=hb[:n, :])
                elif lo >= HALF:
                    nc.sync.dma_start(out=hshardB[lo - HALF:lo - HALF + n, :], in_=hb[:n, :])
                else:
                    nA = HALF - lo
                    nc.sync.dma_start(out=hshardA[lo:HALF, :], in_=hb[:nA, :])
                    nc.sync.dma_start(out=hshardB[0:lo + n - HALF, :], in_=hb[nA:n, :])
                pst = psnode.tile([P, P], f32, tag="pn")
                nc.tensor.transpose(out=pst[:], in_=hb[:], identity=ident_t[:])
                hT = npool.tile([P, P], f32, tag="hT")
                nc.vector.tensor_copy(out=hT[:], in_=pst[:])
                ps2 = psnode.tile([P, P], f32, tag="pn")
                nc.tensor.matmul(out=ps2[:], lhsT=hT[:], rhs=wcat_t[:], start=True, stop=True)
                pre2 = npool.tile([P, P], f32, tag="pre2")
                nc.vector.tensor_tensor(out=pre2[:], in0=ps2[:], in1=bias2_t[:], op=Alu.add)
                nc.scalar.activation(selfbuf2[b][:], pre2[:], Relu)

            if "l1" in phases:
                edge_pass(1, xrows_d, we1_t, W1_t, l1_block_done, offs_t)

            # ---- phase 3: all-gather T2 ----
            if "ag" in phases:
                nc.gpsimd.collective_compute(
                    "AllGather", mybir.AluOpType.bypass,
                    replica_groups=[list(range(N_CORES))],
                    ins=[hshardA[:]], outs=[hfull[0:N_CORES * HALF, :]])
                nc.gpsimd.collective_compute(
                    "AllGather", mybir.AluOpType.bypass,
                    replica_groups=[list(range(N_CORES))],
                    ins=[hshardB[:]], outs=[hfull[N_CORES * HALF:, :]])

            # ---- phase 4: layer-2/3 edge pass ----
            def l2_block_done(b, agg):
                u = npool.tile([P, P], f32, tag="u2")
                nc.vector.tensor_scalar(out=u[:], in0=agg[:],
                                        scalar1=dinvcol_t[:, b:b + 1], scalar2=None,
                                        op0=Alu.mult)
                v = npool.tile([P, P], f32, tag="v2")
                nc.vector.tensor_scalar(out=v[:], in0=selfbuf2[b][:],
                                        scalar1=rdegc_t[:, b:b + 1], scalar2=None,
                                        op0=Alu.mult)
                w = npool.tile([P, P], f32, tag="w2")
                nc.vector.tensor_tensor(out=w[:], in0=u[:], in1=v[:], op=Alu.add)
                lo = b * P
                n = min(P, SHARD - lo)
                nc.sync.dma_start(out=out_d[lo:lo + n, :], in_=w[:n, :])

            if "l2" in phases:
                edge_pass(2, hfull, we2_t, wcat_t, l2_block_done, offs2_t)
            else:
                ztmp = npool.tile([P, P], f32, tag="ztmp")
                nc.vector.memset(ztmp[:], 0.0)
                nc.sync.dma_start(out=out_d[0:P, :], in_=ztmp[:])

    nc.finalize()
    return nc


_CACHE = {}


def kernel(**inputs):
    from concourse.bass_utils import run_bass_kernel_spmd

    per_core, n_chunks, NCH = _host_prep(**inputs)
    key = (tuple(n_chunks), NCH)
    if key not in _CACHE:
        _CACHE[key] = _build_nc(n_chunks, NCH)
    nc = _CACHE[key]
    r = None
    for attempt in range(3):
        try:
            r = run_bass_kernel_spmd(nc, per_core, list(range(N_CORES)))
            break
        except Exception:
            if attempt == 2:
                raise
            import time as _time
            _time.sleep(5.0)
    outs = [r.results[c]["out"] for c in range(N_CORES)]
    full = np.concatenate(outs, axis=0)
    mu = np.ascontiguousarray(full[:, :OUT_F])
    logstd = np.ascontiguousarray(full[:, OUT_F:])
    return (mu, logstd)

